# revision 19
# baseline (speedup 1.0000x reference)
"""Trainium2 Bass kernel for nn_Decoder_GCNMOE (GRU decoder + dense-blend MoE).

Strategy (8 NeuronCores, pure data parallel over batch):
  - batch 128 -> 16 rows per core; each core runs the full network on its slice.
  - GRU (4 layers x 384 steps, the serial bottleneck): slot-major wave
    schedule — the 4 active (layer, chunk) bodies are emitted step-slot by
    step-slot so the Tile scheduler overlaps their dependency chains across
    engines.  Per-step critical chain is trimmed to
       matmul(gh, PSUM preloaded with gi_rz via an identity matmul)
       -> sigmoid -> r*h_n -> +gi_n -> tanh -> v*n -> h = w - (v-1)*h_prev
    using sigmoid(-x) = 1-u (z-gate weight rows negated host-side) so the
    update-gate complement is free, and (v-1)*h_prev fused off-chain with
    scalar_tensor_tensor.  Hidden state lives purely in bf16 (contractive
    recurrence keeps the rounding noise bounded).
  - gi for layers 1..3 is bulk-matmul'd per 16-step chunk from the previous
    layer's history ring; layer 0's input collapses to u_t = u_base +
    t*u_time, built per chunk with 2 small vector ops and run through the
    same bulk-gi path.
  - MoE (49152 tokens): feature-major activations, experts blended by scaling
    the layer input per-expert with the gate weights and accumulating all
    expert matmuls into one PSUM bank.

Assumes (guaranteed by the input spec): mask is all ones, biases
(emb_b, gru_b*, g*_b, beta*) are zeros, lengths is filled with T.
"""

import numpy as np

# ---------------------------------------------------------------- constants
BS, T, D, H, NCLS, E, NJ, NF = 128, 384, 256, 256, 12, 4, 24, 6
MOE_H = 512
OUTD = NJ * NF            # 144
NCORES = 8
B = BS // NCORES          # 16 batch rows per core
KT = H // 128             # 2 k-tiles over H
OT = 3 * H // 128         # 6 o-tiles over 3H
L = 4                     # GRU layers
CH = 16                   # gi-chunk length in steps
TOKC = 512                # MoE tokens per chunk

_STATE = {}


def _bf16(x):
    import ml_dtypes
    return np.asarray(x, dtype=ml_dtypes.bfloat16)


def _f32(x):
    return np.ascontiguousarray(np.asarray(x, dtype=np.float32))


# ------------------------------------------------------------ device program
def _build_nc(T_=T, debug=False):
    import concourse.bass as bass
    import concourse.mybir as mybir
    import concourse.tile as tile

    f32, bf16 = mybir.dt.float32, mybir.dt.bfloat16
    AF, ALU = mybir.ActivationFunctionType, mybir.AluOpType
    NCH = T_ // CH
    NTOK = B * T_
    NMC = NTOK // TOKC

    nc = bass.Bass()
    dt_in = {}

    def din(name, shape, dt=bf16):
        dt_in[name] = nc.dram_tensor(name, list(shape), dt, kind="ExternalInput")
        return dt_in[name]

    whh_t = din("whh_t", [128, L * KT * OT * 128])
    wih_t = din("wih_t", [128, L * KT * OT * 128])   # all 4 layers (l=0 incl)
    emb_t = din("emb_t", [128, 3 * 2 * 128], f32)
    za_t = din("za_t", [128, 3 * B], f32)
    utp_t = din("utp", [128, KT * B], f32)           # time-direction embedding
    tt_t = din("tt", [128, T_], f32)                 # iota 0..T-1
    idmb_t = din("idmb", [128, 128])                 # bf16 identity
    g0t = din("g0t", [128, KT * 4 * 128])
    g1t = din("g1t", [128, 4 * 4 * 128])
    g2r = din("g2r", [128, 4 * 4])
    a0t = din("a0t", [128, E * 2 * 4 * 128])
    a1t = din("a1t", [128, E * 4 * 4 * 128])
    a2ta = din("a2ta", [128, E * 4 * 128])
    a2tb = din("a2tb", [128, E * 4 * 16])
    idm = din("idm", [128, 128], f32)
    out_d = nc.dram_tensor("out", [OUTD, T_, B], f32, kind="ExternalOutput")
    if debug:
        dbg = {n: nc.dram_tensor(n, sh, dt, kind="ExternalOutput") for n, sh, dt in [
            ("dbg_uT", [128, KT * B], bf16),
            ("dbg_zf", [128, KT * B * T_], bf16),
            ("dbg_gi1", [128, 2 * OT * CH * B], bf16),
            ("dbg_gw", [128, 16], f32), ("dbg_g0o", [128, 4 * 512], bf16),
            ("dbg_h1o", [128, 4 * 512], bf16)]}

    with tile.TileContext(nc) as tc:
        with (
            tc.tile_pool(name="wpool", bufs=1) as wp,      # resident weights
            tc.tile_pool(name="state", bufs=1) as sp,      # persistent activations
            tc.tile_pool(name="work", bufs=3) as wk,       # small rotating tiles
        ):
            # ---- resident weight tiles
            def load(name, dram, shape, dt=bf16):
                t_ = wp.tile(shape, dt, tag=name)
                nc.sync.dma_start(t_[:], dram[:])
                return t_

            _whh = load("whh", whh_t, [128, L * KT * OT * 128])
            _wih = load("wih", wih_t, [128, L * KT * OT * 128])
            _emb = load("emb", emb_t, [128, 3 * 2 * 128], f32)
            _za = load("za", za_t, [128, 3 * B], f32)
            _utp = load("utp", utp_t, [128, KT * B], f32)
            _tt = load("tt", tt_t, [128, T_], f32)
            _idmb = load("idmb", idmb_t, [128, 128])
            _g0 = load("g0", g0t, [128, KT * 4 * 128])
            _g1 = load("g1", g1t, [128, 4 * 4 * 128])
            _g2r = load("g2r", g2r, [128, 4 * 4])
            _a0 = load("a0", a0t, [128, E * 2 * 4 * 128])
            _a1 = load("a1", a1t, [128, E * 4 * 4 * 128])
            _a2a = load("a2a", a2ta, [128, E * 4 * 128])
            _a2b = load("a2b", a2tb, [128, E * 4 * 16])

            def _sl(tile_, idx, c):
                return tile_[:, idx * c:(idx + 1) * c]

            w_whh = lambda l, kt, ot: _sl(_whh, (l * KT + kt) * OT + ot, 128)
            w_wih = lambda l, kt, ot: _sl(_wih, (l * KT + kt) * OT + ot, 128)
            w_emb = lambda kt, mt: _sl(_emb, kt * 2 + mt, 128)
            w_za = lambda kt: _sl(_za, kt, B)
            w_g0 = lambda kt, mt: _sl(_g0, kt * 4 + mt, 128)
            w_g1 = lambda kt, mt: _sl(_g1, kt * 4 + mt, 128)
            w_g2r = lambda kt: _sl(_g2r, kt, 4)
            w_a0 = lambda e, kt, mt: _sl(_a0, (e * 2 + kt) * 4 + mt, 128)
            w_a1 = lambda e, kt, mt: _sl(_a1, (e * 4 + kt) * 4 + mt, 128)
            w_a2a = lambda e, kt: _sl(_a2a, e * 4 + kt, 128)
            w_a2b = lambda e, kt: _sl(_a2b, e * 4 + kt, 16)
            w_idm = load("idm", idm, [128, 128], f32)

            ones1 = sp.tile([1, 128], bf16, tag="ones1")
            nc.gpsimd.memset(ones1[:], 1.0)
            zero_h = sp.tile([128, 2 * B], bf16, tag="zero_h")
            nc.gpsimd.memset(zero_h[:], 0.0)

            # persistent activation state
            hist = [sp.tile([128, 32 * 2 * B], bf16, tag=f"hist{l}", name=f"hist{l}")
                    for l in range(3)]
            zfT = sp.tile([128, KT * NTOK], bf16, tag="zfT")
            # per-layer gi buffers, layout (ot, s, b); single-buffered —
            # the chunk-boundary WAR serializes bulk-copy vs last readers.
            gi_sb = [sp.tile([128, OT * CH * B], bf16, tag=f"gi{l}",
                             name=f"gi{l}") for l in range(L)]
            # gate weights for all MoE chunks, [4 experts, NMC*TOKC]
            gwTall = sp.tile([4, NMC * TOKC], bf16, tag="gwTall")

            # ------------------------------------------------ embedding -> uT
            with tc.tile_pool(name="ps_pre", bufs=2, space="PSUM") as ppre:
                uT = sp.tile([128, KT * B], bf16, tag="uT")
                for mt in range(2):
                    pu = ppre.tile([128, B], f32, tag="pu")
                    for kt in range(3):
                        nc.tensor.matmul(pu[:], w_emb(kt, mt), w_za(kt),
                                         start=(kt == 0), stop=(kt == 2))
                    nc.scalar.copy(uT[:, mt * B:(mt + 1) * B], pu[:])

            # ------------------------------------------------ GRU waves
            with tc.tile_pool(name="ps_pg", bufs=1, space="PSUM") as ppg, \
                 tc.tile_pool(name="ps_gib", bufs=2, space="PSUM") as pgib, \
                 tc.tile_pool(name="ps_gg", bufs=2, space="PSUM") as pgg, \
                 tc.tile_pool(name="gmoe", bufs=2) as mgg:

                pg = [ppg.tile([128, OT * B], f32, tag=f"pg{l}", name=f"pg{l}")
                      for l in range(L)]

                def h_src(l, t, kt):
                    if t < 0:
                        return zero_h[:, kt * B:(kt + 1) * B]
                    if l < 3:
                        s = (t % 32) * 2 * B + kt * B
                        return hist[l][:, s:s + B]
                    return zfT[:, kt * NTOK + t * B: kt * NTOK + t * B + B]

                def h_prev_full(l, t):
                    # [128, 2, B] view of h_{t} (bf16)
                    if t < 0:
                        return zero_h[:].rearrange("p (k b) -> p k b", b=B)
                    if l < 3:
                        s = (t % 32) * 2 * B
                        return hist[l][:, s:s + 2 * B].rearrange(
                            "p (k b) -> p k b", b=B)
                    return zfT[:].rearrange("p (k n) -> p k n", k=KT)[
                        :, :, t * B:(t + 1) * B]

                def h_dst(l, t):
                    if l < 3:
                        s = (t % 32) * 2 * B
                        return hist[l][:, s:s + 2 * B].rearrange(
                            "p (k b) -> p k b", b=B)
                    return zfT[:].rearrange("p (k n) -> p k n", k=KT)[
                        :, :, t * B:(t + 1) * B]

                def emit_bulk_gi(l, c):
                    """Compute gi for (layer l, chunk c) into gi_sb[l];
                    src is layer l-1 history (or u_t for l=0)."""
                    if l == 0:
                        # u_chunk[s, kt, b] = uT + t*utp  (2 DVE ops)
                        usc = wk.tile([128, CH * 2 * B], f32, tag="usc")
                        u3 = usc[:].rearrange("p (s z) -> p s z", z=2 * B)
                        tts = _tt[:, c * CH:(c + 1) * CH].unsqueeze(
                            2).broadcast_to((128, CH, 2 * B))
                        utv = _utp[:].unsqueeze(1).broadcast_to(
                            (128, CH, 2 * B))
                        nc.vector.tensor_tensor(u3, tts, utv, op=ALU.mult)
                        ub = wk.tile([128, CH * 2 * B], bf16, tag="ub")
                        ub3 = ub[:].rearrange("p (s z) -> p s z", z=2 * B)
                        ubv = uT[:].unsqueeze(1).broadcast_to((128, CH, 2 * B))
                        nc.vector.tensor_tensor(ub3, u3, ubv, op=ALU.add)
                        src = ub3
                    else:
                        src = hist[l - 1][:].rearrange(
                            "p (s z) -> p s z", z=2 * B)
                    s0 = 0 if l == 0 else (c * CH) % 32
                    for otp in range(3):          # ot pairs
                        pb = pgib.tile([128, 2 * CH * B], f32, tag="gib")
                        for oti in range(2):
                            ot = otp * 2 + oti
                            for kt in range(KT):
                                rhs = src[:, s0:s0 + CH, kt * B:(kt + 1) * B]
                                nc.tensor.matmul(
                                    pb[:, oti * CH * B:(oti + 1) * CH * B],
                                    w_wih(l, kt, ot), rhs, start=(kt == 0),
                                    stop=(kt == KT - 1))
                        dst = gi_sb[l][:, otp * 2 * CH * B:
                                       (otp + 1) * 2 * CH * B]
                        if otp == 0:
                            nc.scalar.copy(dst, pb[:])
                        else:
                            nc.vector.tensor_copy(dst, pb[:])

                def step_body(l, c, s):
                    t = c * CH + s
                    g = gi_sb[l][:].rearrange(
                        "p (o s b) -> p o s b", o=OT, s=CH)
                    gi_rz = g[:, 0:4, s, :]
                    gi_n = g[:, 4:6, s, :]
                    pgl = pg[l]
                    # preload gi_rz into PSUM, accumulate gh on top
                    nc.tensor.matmul(pgl[:, 0:4 * B], _idmb[:], gi_rz,
                                     start=True, stop=False,
                                     skip_group_check=True)
                    for ot in range(4):
                        for kt in range(KT):
                            nc.tensor.matmul(
                                pgl[:, ot * B:(ot + 1) * B],
                                w_whh(l, kt, ot), h_src(l, t - 1, kt),
                                start=False, stop=(kt == KT - 1),
                                skip_group_check=True)
                    for ot in (4, 5):
                        for kt in range(KT):
                            nc.tensor.matmul(
                                pgl[:, ot * B:(ot + 1) * B],
                                w_whh(l, kt, ot), h_src(l, t - 1, kt),
                                start=(kt == 0), stop=(kt == KT - 1),
                                skip_group_check=True)
                    # r | v = sigmoid(rz)   (z rows negated => v = 1-u)
                    ru = wk.tile([128, 4 * B], f32, tag=f"ru{l}",
                                 name=f"ru{l}_{t}")
                    nc.scalar.activation(ru[:], pgl[:, 0:4 * B], AF.Sigmoid)
                    # hn = r * gh_n
                    hn = wk.tile([128, 2 * B], f32, tag=f"hn{l}",
                                 name=f"hn{l}_{t}")
                    nc.vector.tensor_tensor(hn[:], ru[:, 0:2 * B],
                                            pgl[:, 4 * B:6 * B], op=ALU.mult)
                    # n_in = hn + gi_n
                    n_in = wk.tile([128, 2 * B], f32, tag=f"ni{l}",
                                   name=f"ni{l}_{t}")
                    nc.gpsimd.tensor_tensor(
                        n_in[:].rearrange("p (o b) -> p o b", b=B),
                        hn[:].rearrange("p (o b) -> p o b", b=B),
                        gi_n, op=ALU.add)
                    nt = wk.tile([128, 2 * B], f32, tag=f"nt{l}",
                                 name=f"nt{l}_{t}")
                    nc.scalar.activation(nt[:], n_in[:], AF.Tanh)
                    # hmv = (v-1)*h_prev   (off the nt-chain)
                    hmv = wk.tile([128, 2 * B], f32, tag=f"hm{l}",
                                  name=f"hm{l}_{t}")
                    nc.vector.scalar_tensor_tensor(
                        hmv[:].rearrange("p (k b) -> p k b", b=B),
                        ru[:, 2 * B:4 * B].rearrange("p (k b) -> p k b", b=B),
                        1.0, h_prev_full(l, t - 1),
                        op0=ALU.subtract, op1=ALU.mult)
                    # w = v*nt ; h = w - hmv  -> bf16 state
                    w_ = wk.tile([128, 2 * B], f32, tag=f"w{l}",
                                 name=f"w{l}_{t}")
                    nc.vector.tensor_tensor(w_[:], ru[:, 2 * B:4 * B], nt[:],
                                            op=ALU.mult)
                    nc.gpsimd.tensor_tensor(
                        h_dst(l, t),
                        w_[:].rearrange("p (k b) -> p k b", b=B),
                        hmv[:].rearrange("p (k b) -> p k b", b=B),
                        op=ALU.subtract)

                for w in range(NCH + L - 1):
                    active = [(l, w - l) for l in range(L) if 0 <= w - l < NCH]
                    for (l, c) in active:
                        emit_bulk_gi(l, c)
                    for s in range(CH):
                        for (l, c) in active:
                            step_body(l, c, s)

                # ---- MoE gating towers: emitted last (lowest priority) so
                # they fill engine idle during the GRU waves; zfT data deps
                # gate each tower on its tokens being ready.
                def elu1g(dst, src_ps):
                    tr = wk.tile([128, TOKC], f32, tag="elu_r")
                    nc.scalar.activation(tr[:], src_ps, AF.Relu)
                    te = wk.tile([128, TOKC], f32, tag="elu_e")
                    nc.scalar.activation(te[:], src_ps, AF.Exp)
                    nc.vector.tensor_scalar(te[:], te[:], 1.0, -1.0,
                                            op0=ALU.min, op1=ALU.add)
                    nc.vector.tensor_tensor(dst, tr[:], te[:], op=ALU.add)

                for ctk in range(NMC):
                    def zcg(kt):
                        return zfT[:, kt * NTOK + ctk * TOKC:
                                   kt * NTOK + (ctk + 1) * TOKC]
                    g0o = mgg.tile([128, 4 * TOKC], bf16, tag="g0o")
                    for mt in range(4):
                        pb = pgg.tile([128, TOKC], f32, tag="gg")
                        for kt in range(KT):
                            nc.tensor.matmul(pb[:], w_g0(kt, mt), zcg(kt),
                                             start=(kt == 0), stop=(kt == KT - 1))
                        elu1g(g0o[:, mt * TOKC:(mt + 1) * TOKC], pb[:])
                    g1o = mgg.tile([128, 4 * TOKC], bf16, tag="g1o")
                    for mt in range(4):
                        pb = pgg.tile([128, TOKC], f32, tag="gg")
                        for kt in range(4):
                            nc.tensor.matmul(pb[:], w_g1(kt, mt),
                                             g0o[:, kt * TOKC:(kt + 1) * TOKC],
                                             start=(kt == 0), stop=(kt == 3))
                        elu1g(g1o[:, mt * TOKC:(mt + 1) * TOKC], pb[:])
                    pg2 = pgg.tile([128, 16], f32, tag="gg", name=f"pg2_{ctk}")
                    for tt in range(4):
                        for kt in range(4):
                            nc.tensor.matmul(
                                pg2[:, tt * 4:(tt + 1) * 4],
                                g1o[:, kt * TOKC + tt * 128: kt * TOKC + (tt + 1) * 128],
                                w_g2r(kt), start=(kt == 0), stop=(kt == 3),
                                skip_group_check=True)
                    g4 = lambda a: a.rearrange("p (g e) -> p g e", e=4)
                    mx = wk.tile([128, 4], f32, tag="sm_mx")
                    nc.vector.tensor_reduce(mx[:], g4(pg2[:]),
                                            axis=mybir.AxisListType.X,
                                            op=ALU.max)
                    sub = wk.tile([128, 16], f32, tag="sm_sub")
                    nc.vector.tensor_tensor(g4(sub[:]), g4(pg2[:]),
                                            mx[:].unsqueeze(2).broadcast_to(
                                                (128, 4, 4)), op=ALU.subtract)
                    ex = wk.tile([128, 16], f32, tag="sm_ex")
                    nc.scalar.activation(ex[:], sub[:], AF.Exp)
                    sm = wk.tile([128, 4], f32, tag="sm_sum")
                    nc.vector.tensor_reduce(sm[:], g4(ex[:]),
                                            axis=mybir.AxisListType.X, op=ALU.add)
                    rec = wk.tile([128, 4], f32, tag="sm_rec")
                    nc.vector.reciprocal(rec[:], sm[:])
                    gw = wk.tile([128, 16], f32, tag="sm_gw")
                    nc.vector.tensor_tensor(g4(gw[:]), g4(ex[:]),
                                            rec[:].unsqueeze(2).broadcast_to(
                                                (128, 4, 4)), op=ALU.mult)
                    pgt = pgg.tile([4, TOKC], f32, tag="gg", name=f"pgt_{ctk}")
                    for tt in range(4):
                        nc.tensor.transpose(pgt[:, tt * 128:(tt + 1) * 128],
                                            gw[:, tt * 4:(tt + 1) * 4], w_idm[:])
                    nc.scalar.copy(gwTall[:, ctk * TOKC:(ctk + 1) * TOKC],
                                   pgt[:])
                    if debug and ctk == 0:
                        nc.sync.dma_start(dbg["dbg_gw"][:], gw[:])
                        nc.sync.dma_start(dbg["dbg_g0o"][:], g0o[:])

            if debug:
                nc.sync.dma_start(dbg["dbg_uT"][:], uT[:])
                nc.sync.dma_start(dbg["dbg_zf"][:], zfT[:])
                nc.sync.dma_start(dbg["dbg_gi1"][:], gi_sb[1][:])

            # ------------------------------------------------ MoE chunks
            with tc.tile_pool(name="ps_moe", bufs=4, space="PSUM") as pmo, \
                 tc.tile_pool(name="ps_sm", bufs=2, space="PSUM") as psm, \
                 tc.tile_pool(name="ps_bc", bufs=2, space="PSUM") as pbp, \
                 tc.tile_pool(name="moeg", bufs=2) as mpg, \
                 tc.tile_pool(name="moe", bufs=1) as mp, \
                 tc.tile_pool(name="moesc", bufs=2) as msc:

                def elu1(dst, src_ps):
                    # elu(x) = relu(x) + (min(exp(x), 1) - 1); f32 until the
                    # final add.
                    tr = wk.tile([128, TOKC], f32, tag="elu_r")
                    nc.scalar.activation(tr[:], src_ps, AF.Relu)
                    te = wk.tile([128, TOKC], f32, tag="elu_e")
                    nc.scalar.activation(te[:], src_ps, AF.Exp)
                    nc.vector.tensor_scalar(te[:], te[:], 1.0, -1.0,
                                            op0=ALU.min, op1=ALU.add)
                    nc.vector.tensor_tensor(dst, tr[:], te[:], op=ALU.add)

                def zc(ctk, kt):
                    return zfT[:, kt * NTOK + ctk * TOKC:
                               kt * NTOK + (ctk + 1) * TOKC]

                def emit_broadcast(ctk):
                    """Broadcast the stored gate weights for chunk ctk to all
                    128 partitions: gwb[p, e*TOKC+n] = gw[n, e]."""
                    gwf = mpg.tile([1, E * TOKC], bf16, tag="gwf")
                    nc.sync.dma_start(
                        gwf[:].rearrange("p (e n) -> p e n", e=E),
                        gwTall[:, ctk * TOKC:(ctk + 1) * TOKC])
                    gwb = mpg.tile([128, E * TOKC], bf16, tag="gwb")
                    for e in range(E):
                        pbc = pbp.tile([128, TOKC], f32, tag="bc",
                                       name=f"pbc{ctk}_{e}")
                        nc.tensor.matmul(pbc[:], ones1[:],
                                         gwf[0:1, e * TOKC:(e + 1) * TOKC],
                                         start=True, stop=True)
                        nc.scalar.copy(gwb[:, e * TOKC:(e + 1) * TOKC], pbc[:])
                    return gwb

                def emit_blends(ctk, gwb):
                    # blend 0: inputs zc (2 k-tiles), out 512
                    pbs = [pmo.tile([128, TOKC], f32, tag="big", name=f"pbs{_i}")
                           for _i in range(4)]
                    xsc = msc.tile([128, KT * TOKC], bf16, tag="hsc")
                    for e in range(E):
                        for kt in range(KT):
                            nc.vector.tensor_tensor(
                                xsc[:, kt * TOKC:(kt + 1) * TOKC], zc(ctk, kt),
                                gwb[:, e * TOKC:(e + 1) * TOKC], op=ALU.mult)
                        for mt in range(4):
                            for kt in range(KT):
                                nc.tensor.matmul(
                                    pbs[mt][:], w_a0(e, kt, mt),
                                    xsc[:, kt * TOKC:(kt + 1) * TOKC],
                                    start=(e == 0 and kt == 0),
                                    stop=(e == 3 and kt == KT - 1),
                                    skip_group_check=True)
                    h1o = mp.tile([128, 4 * TOKC], bf16, tag="h1o")
                    for mt in range(4):
                        elu1(h1o[:, mt * TOKC:(mt + 1) * TOKC], pbs[mt][:])
                    if debug and ctk == 0:
                        nc.sync.dma_start(dbg["dbg_h1o"][:], h1o[:])

                    # blend 1: inputs h1o (4 k-tiles)
                    pbs = [pmo.tile([128, TOKC], f32, tag="big", name=f"pbs{_i}")
                           for _i in range(4)]
                    h1sc = msc.tile([128, 4 * TOKC], bf16, tag="hsc")
                    for e in range(E):
                        for kt in range(4):
                            nc.vector.tensor_tensor(
                                h1sc[:, kt * TOKC:(kt + 1) * TOKC],
                                h1o[:, kt * TOKC:(kt + 1) * TOKC],
                                gwb[:, e * TOKC:(e + 1) * TOKC], op=ALU.mult)
                        for mt in range(4):
                            for kt in range(4):
                                nc.tensor.matmul(
                                    pbs[mt][:], w_a1(e, kt, mt),
                                    h1sc[:, kt * TOKC:(kt + 1) * TOKC],
                                    start=(e == 0 and kt == 0),
                                    stop=(e == 3 and kt == 3),
                                    skip_group_check=True)
                    h2o = mp.tile([128, 4 * TOKC], bf16, tag="h2o")
                    for mt in range(4):
                        elu1(h2o[:, mt * TOKC:(mt + 1) * TOKC], pbs[mt][:])
                    # blend 2: out 144 = 128 + 16
                    poa = pmo.tile([128, TOKC], f32, tag="big")
                    pob = psm.tile([16, TOKC], f32, tag="sm", name=f"pob_{ctk}")
                    h2sc = msc.tile([128, 4 * TOKC], bf16, tag="hsc")
                    for e in range(E):
                        for kt in range(4):
                            nc.vector.tensor_tensor(
                                h2sc[:, kt * TOKC:(kt + 1) * TOKC],
                                h2o[:, kt * TOKC:(kt + 1) * TOKC],
                                gwb[:, e * TOKC:(e + 1) * TOKC], op=ALU.mult)
                        for kt in range(4):
                            last = (e == 3 and kt == 3)
                            nc.tensor.matmul(poa[:], w_a2a(e, kt),
                                             h2sc[:, kt * TOKC:(kt + 1) * TOKC],
                                             start=(e == 0 and kt == 0), stop=last,
                                             skip_group_check=True)
                            nc.tensor.matmul(pob[:], w_a2b(e, kt),
                                             h2sc[:, kt * TOKC:(kt + 1) * TOKC],
                                             start=(e == 0 and kt == 0), stop=last,
                                             skip_group_check=True)
                    oa = mp.tile([128, TOKC], f32, tag="oa")
                    nc.scalar.copy(oa[:], poa[:])
                    ob = mp.tile([16, TOKC], f32, tag="ob")
                    nc.scalar.copy(ob[:], pob[:])
                    # out[o, t, b]: src [o_part, (t 32, b 16)] - both contiguous
                    t0 = ctk * (TOKC // B)
                    nc.sync.dma_start(out_d[0:128, t0:t0 + 32, :], oa[:].rearrange(
                        "p (t b) -> p t b", b=B))
                    nc.sync.dma_start(out_d[128:144, t0:t0 + 32, :], ob[:].rearrange(
                        "p (t b) -> p t b", b=B))

                gwb_cur = emit_broadcast(0)
                for ctk in range(NMC):
                    gwb_next = emit_broadcast(ctk + 1) if ctk + 1 < NMC else None
                    emit_blends(ctk, gwb_cur)
                    gwb_cur = gwb_next
    return nc


# ------------------------------------------------------------- walrus fixup
def _fix_sync_waits(nc, max_waits=1):
    """This walrus build allows only one sync wait per instruction; move
    excess waits onto NOPs inserted ahead of the instruction."""
    import concourse.mybir as mybir
    import bass_rust
    ctr = 0
    for f in nc.m.functions:
        for blk in f.blocks:
            out = []
            changed = False
            for inst in blk.instructions:
                si = inst.sync_info
                if si is not None and si.on_wait and len(si.on_wait) > max_waits:
                    waits = list(si.on_wait)
                    extra, keep = waits[:-max_waits], waits[-max_waits:]
                    for w_ in extra:
                        ctr += 1
                        nop = mybir.InstNoOp(name=f"WSPLIT-{ctr}", ins=[], outs=[])
                        nop.engine = inst.engine
                        nop.sync_info = bass_rust.SyncInfo(on_wait=[w_], on_update=[])
                        out.append(nop)
                    inst.sync_info = bass_rust.SyncInfo(
                        on_wait=keep, on_update=list(si.on_update))
                    changed = True
                out.append(inst)
            if changed:
                blk.instructions = out
    return ctr


# ------------------------------------------------------------- preprocessing
def _prep_core_inputs(inputs, T_=T):
    z = _f32(inputs["z"])
    y = np.asarray(inputs["y"]).astype(np.int64)
    lengths = np.asarray(inputs["lengths"]).astype(np.float64)
    emb_w = _f32(inputs["emb_w"])      # [H, D+NC+1]
    gru_wih = _f32(inputs["gru_wih"]).copy()  # [4, 3H, H]
    gru_whh = _f32(inputs["gru_whh"]).copy()
    g0_w = _f32(inputs["g0_w"]); g1_w = _f32(inputs["g1_w"]); g2_w = _f32(inputs["g2_w"])
    g0_b = _f32(inputs["g0_b"]); g1_b = _f32(inputs["g1_b"]); g2_b = _f32(inputs["g2_b"])
    a0 = _f32(inputs["alpha0"]); a1 = _f32(inputs["alpha1"]); a2 = _f32(inputs["alpha2"])
    b0 = _f32(inputs["beta0"]); b1 = _f32(inputs["beta1"]); b2 = _f32(inputs["beta2"])
    emb_b = _f32(inputs["emb_b"])
    bsum = _f32(inputs["gru_bih"]) + _f32(inputs["gru_bhh"])  # [4, 3H] assumed zero

    # negate z-gate rows so sigmoid yields v = 1-u directly
    gru_wih[:, H:2 * H, :] *= -1.0
    gru_whh[:, H:2 * H, :] *= -1.0

    # ---- shared (replicated) tensors
    def pack_lhsT(w, cols=128):
        # w: [O, K]; lhsT = w.T tiled [K//128, O//cols, 128, cols]
        # -> flat [128, ntiles*cols], tile index = kt*OT_ + ot (kt-major)
        O, K = w.shape
        ktn, otn = K // 128, O // cols
        wt = np.ascontiguousarray(w.T).reshape(ktn, 128, otn, cols)
        return wt.transpose(1, 0, 2, 3).reshape(128, ktn * otn * cols)

    whh_t = _bf16(np.concatenate([pack_lhsT(gru_whh[l]) for l in range(4)], axis=1))
    wih_t = _bf16(np.concatenate([pack_lhsT(gru_wih[l]) for l in range(4)], axis=1))
    embT = np.zeros((256, 384), np.float32)
    embT[:, :269] = emb_w
    emb_t = _f32(pack_lhsT(embT))                 # [128, 3kt*2mt*128]
    g0t = _bf16(pack_lhsT(g0_w))
    g1t = _bf16(pack_lhsT(g1_w))
    # g2 rhs tiles: g2_w.T [512, 4] -> [4kt][128, 4] -> [128, 16]
    g2r = _bf16(np.ascontiguousarray(g2_w.T).reshape(4, 128, 4)
                .transpose(1, 0, 2).reshape(128, 16))
    a0t = _bf16(np.concatenate([pack_lhsT(a0[e]) for e in range(E)], axis=1))
    a1t = _bf16(np.concatenate([pack_lhsT(a1[e]) for e in range(E)], axis=1))
    # alpha2: [E, 144, 512]: lhsT [512, 144] -> cols 0:128 and 128:144
    a2T = np.stack([np.ascontiguousarray(a2[e].T) for e in range(E)])  # [E,512,144]
    a2r = a2T.reshape(E, 4, 128, 144)
    a2ta = _bf16(a2r[:, :, :, 0:128].transpose(2, 0, 1, 3).reshape(128, E * 4 * 128))
    a2tb = _bf16(a2r[:, :, :, 128:144].transpose(2, 0, 1, 3).reshape(128, E * 4 * 16))
    idm = _f32(np.eye(128))
    idmb = _bf16(np.eye(128))
    tt = _f32(np.broadcast_to(np.arange(T_, dtype=np.float32), (128, T_)))
    # NOTE: biases (g0_b/g1_b/g2_b, emb_b, gru biases) and betas are zero by
    # the input spec; they are asserted here so a nonzero case fails loudly.
    for _nm, _v in [("g0_b", g0_b), ("g1_b", g1_b), ("g2_b", g2_b),
                    ("emb_b", emb_b), ("beta0", b0), ("beta1", b1),
                    ("beta2", b2), ("gru_b", bsum)]:
        assert np.abs(_v).max() == 0.0, f"{_nm} nonzero; kernel assumes zero"

    # ---- per-core tensors
    yoh = np.zeros((BS, NCLS), np.float32)
    yoh[np.arange(BS), y] = 1.0
    za = np.concatenate([z, yoh], axis=1)              # [BS, 268]
    za_pad = np.zeros((BS, 384), np.float32)
    za_pad[:, :268] = za
    # feature 268 (time) enters via utp, not za.
    cb = 1.0 / (lengths - 1.0)
    u_time = emb_w[:, 268]                             # [256]

    maps = []
    for cidx in range(NCORES):
        sl = slice(cidx * B, (cidx + 1) * B)
        za_c = np.ascontiguousarray(za_pad[sl].T)      # [384, B]
        za_t = _f32(za_c.reshape(3, 128, B).transpose(1, 0, 2).reshape(128, 3 * B))
        # utp[p, kt, b] = u_time[kt*128+p] * cb[b]
        utp_v = np.einsum("f,b->fb", u_time, cb[sl])   # [256, B]
        utp = _f32(utp_v.reshape(2, 128, B).transpose(1, 0, 2).reshape(128, 2 * B))
        maps.append(dict(
            whh_t=whh_t, wih_t=wih_t, emb_t=emb_t, za_t=za_t,
            utp=utp, tt=tt, idmb=idmb, g0t=g0t, g1t=g1t, g2r=g2r,
            a0t=a0t, a1t=a1t, a2ta=a2ta, a2tb=a2tb, idm=idm,
        ))
    return maps


# ------------------------------------------------------------------ runner
def _get_runner(T_=T):
    key = ("runner", T_)
    if key in _STATE:
        return _STATE[key]
    import jax
    from jax.sharding import Mesh, PartitionSpec
    try:
        from jax.experimental.shard_map import shard_map
    except ImportError:
        from jax.shard_map import shard_map
    import concourse.mybir as mybir
    from concourse import bass2jax

    nc = _build_nc(T_)
    _fix_sync_waits(nc)
    bass2jax.install_neuronx_cc_hook()
    partition_name = nc.partition_id_tensor.name if nc.partition_id_tensor else None
    in_names, out_names, out_avals = [], [], []
    for alloc in nc.m.functions[0].allocations:
        if not isinstance(alloc, mybir.MemoryLocationSet):
            continue
        name = alloc.memorylocations[0].name
        if alloc.kind == "ExternalInput":
            if name != partition_name:
                in_names.append(name)
        elif alloc.kind == "ExternalOutput":
            out_names.append(name)
            out_avals.append(jax.core.ShapedArray(
                tuple(alloc.tensor_shape), mybir.dt.np(alloc.dtype)))
    n_params = len(in_names)
    all_names = in_names + out_names + ([partition_name] if partition_name else [])

    def _body(*args):
        operands = list(args)
        if partition_name is not None:
            operands.append(bass2jax.partition_id_tensor())
        return tuple(bass2jax._bass_exec_p.bind(
            *operands, out_avals=tuple(out_avals), in_names=tuple(all_names),
            out_names=tuple(out_names), lowering_input_output_aliases=(),
            sim_require_finite=True, sim_require_nnan=True, nc=nc))

    devices = jax.devices()[:NCORES]
    mesh = Mesh(np.asarray(devices), ("core",))
    n_outs = len(out_names)
    sharded = jax.jit(
        shard_map(_body, mesh=mesh,
                  in_specs=(PartitionSpec("core"),) * (n_params + n_outs),
                  out_specs=(PartitionSpec("core"),) * n_outs),
        keep_unused=True)
    runner = dict(sharded=sharded, in_names=in_names, out_names=out_names,
                  out_avals=out_avals)
    _STATE[key] = runner
    return runner


def _run_device(maps, T_=T, timing=None):
    import jax
    r = _get_runner(T_)
    concat_in = [np.concatenate([np.asarray(maps[c][n]) for c in range(NCORES)],
                                axis=0) for n in r["in_names"]]
    zeros = [np.zeros((NCORES * a.shape[0], *a.shape[1:]), a.dtype)
             for a in r["out_avals"]]
    din = [jax.device_put(x) for x in concat_in]
    dz = [jax.device_put(z) for z in zeros]
    out = r["sharded"](*din, *dz)
    jax.block_until_ready(out)
    if timing is not None:
        import time
        for _ in range(timing.get("iters", 10)):
            t0 = time.perf_counter()
            out = r["sharded"](*din, *dz)
            jax.block_until_ready(out)
            timing.setdefault("times", []).append(time.perf_counter() - t0)
    o = np.asarray(out[0])
    per_core = o.reshape(NCORES, OUTD, T_, B)
    return per_core


def kernel(**inputs):
    maps = _prep_core_inputs(inputs, T)
    per_core = _run_device(maps, T)          # [NCORES, OUTD, T, B]
    full = per_core.transpose(0, 3, 1, 2)    # [NCORES, B, OUTD, T]
    full = full.reshape(BS, NJ, NF, T)
    return np.ascontiguousarray(full.astype(np.float32))


# revision 29
# speedup vs baseline: 1.0754x; 1.0754x over previous
"""Trainium2 Bass kernel for nn_Decoder_GCNMOE (GRU decoder + dense-blend MoE).

Strategy (8 NeuronCores, pure data parallel over batch):
  - batch 128 -> 16 rows per core; each core runs the full network on its slice.
  - GRU (4 layers x 384 steps, the serial bottleneck): slot-major wave
    schedule — the 4 active (layer, chunk) bodies are emitted step-slot by
    step-slot so the Tile scheduler overlaps their dependency chains across
    engines.  Per-step critical chain is trimmed to
       matmul(gh, PSUM preloaded with gi_rz via an identity matmul)
       -> sigmoid -> r*h_n -> +gi_n -> tanh -> v*n -> h = w - (v-1)*h_prev
    using sigmoid(-x) = 1-u (z-gate weight rows negated host-side) so the
    update-gate complement is free, and (v-1)*h_prev fused off-chain with
    scalar_tensor_tensor.  Hidden state lives purely in bf16 (contractive
    recurrence keeps the rounding noise bounded).
  - gi for layers 1..3 is bulk-matmul'd per 16-step chunk from the previous
    layer's history ring; layer 0's input collapses to u_t = u_base +
    t*u_time, built per chunk with 2 small vector ops and run through the
    same bulk-gi path.
  - MoE (49152 tokens): feature-major activations, experts blended by scaling
    the layer input per-expert with the gate weights and accumulating all
    expert matmuls into one PSUM bank.

Assumes (guaranteed by the input spec): mask is all ones, biases
(emb_b, gru_b*, g*_b, beta*) are zeros, lengths is filled with T.
"""

import numpy as np

# ---------------------------------------------------------------- constants
BS, T, D, H, NCLS, E, NJ, NF = 128, 384, 256, 256, 12, 4, 24, 6
MOE_H = 512
OUTD = NJ * NF            # 144
NCORES = 8
B = BS // NCORES          # 16 batch rows per core
KT = H // 128             # 2 k-tiles over H
OT = 3 * H // 128         # 6 o-tiles over 3H
L = 4                     # GRU layers
CH = 16                   # gi-chunk length in steps
TOKC = 512                # MoE tokens per chunk

_STATE = {}


def _bf16(x):
    import ml_dtypes
    return np.asarray(x, dtype=ml_dtypes.bfloat16)


def _f32(x):
    return np.ascontiguousarray(np.asarray(x, dtype=np.float32))


# ------------------------------------------------------------ device program
def _build_nc(T_=T, debug=False):
    import concourse.bass as bass
    import concourse.mybir as mybir
    import concourse.tile as tile

    f32, bf16 = mybir.dt.float32, mybir.dt.bfloat16
    AF, ALU = mybir.ActivationFunctionType, mybir.AluOpType
    NCH = T_ // CH
    NTOK = B * T_
    NMC = NTOK // TOKC

    nc = bass.Bass()
    dt_in = {}

    def din(name, shape, dt=bf16):
        dt_in[name] = nc.dram_tensor(name, list(shape), dt, kind="ExternalInput")
        return dt_in[name]

    whh_t = din("whh_t", [128, L * KT * OT * 128])
    wih_t = din("wih_t", [128, L * KT * OT * 128])   # all 4 layers (l=0 incl)
    emb_t = din("emb_t", [128, 3 * 2 * 128], f32)
    za_t = din("za_t", [128, 3 * B], f32)
    utp_t = din("utp", [128, KT * B], f32)           # time-direction embedding
    tt_t = din("tt", [128, T_], f32)                 # iota 0..T-1
    idmb_t = din("idmb", [128, 128])                 # bf16 identity
    g0t = din("g0t", [128, KT * 4 * 128])
    g1t = din("g1t", [128, 4 * 4 * 128])
    g2r = din("g2r", [128, 4 * 4])
    a0t = din("a0t", [128, E * 2 * 4 * 128])
    a1t = din("a1t", [128, E * 4 * 4 * 128])
    a2ta = din("a2ta", [128, E * 4 * 128])
    a2tb = din("a2tb", [128, E * 4 * 16])
    idm = din("idm", [128, 128], f32)
    out_d = nc.dram_tensor("out", [OUTD, T_, B], f32, kind="ExternalOutput")
    if debug:
        dbg = {n: nc.dram_tensor(n, sh, dt, kind="ExternalOutput") for n, sh, dt in [
            ("dbg_uT", [128, KT * B], bf16),
            ("dbg_zf", [128, KT * B * T_], bf16),
            ("dbg_gi1", [128, 2 * OT * CH * B], bf16),
            ("dbg_gw", [128, 16], f32), ("dbg_g0o", [128, 4 * 512], bf16),
            ("dbg_h1o", [128, 4 * 512], bf16)]}

    with tile.TileContext(nc) as tc:
        with (
            tc.tile_pool(name="wpool", bufs=1) as wp,      # resident weights
            tc.tile_pool(name="state", bufs=1) as sp,      # persistent activations
            tc.tile_pool(name="work", bufs=3) as wk,       # small rotating tiles
        ):
            # ---- resident weight tiles
            def load(name, dram, shape, dt=bf16):
                t_ = wp.tile(shape, dt, tag=name)
                nc.sync.dma_start(t_[:], dram[:])
                return t_

            _whh = load("whh", whh_t, [128, L * KT * OT * 128])
            _wih = load("wih", wih_t, [128, L * KT * OT * 128])
            _emb = load("emb", emb_t, [128, 3 * 2 * 128], f32)
            _za = load("za", za_t, [128, 3 * B], f32)
            _utp = load("utp", utp_t, [128, KT * B], f32)
            _tt = load("tt", tt_t, [128, T_], f32)
            _idmb = load("idmb", idmb_t, [128, 128])
            _g0 = load("g0", g0t, [128, KT * 4 * 128])
            _g1 = load("g1", g1t, [128, 4 * 4 * 128])
            _g2r = load("g2r", g2r, [128, 4 * 4])
            _a0 = load("a0", a0t, [128, E * 2 * 4 * 128])
            _a1 = load("a1", a1t, [128, E * 4 * 4 * 128])
            _a2a = load("a2a", a2ta, [128, E * 4 * 128])
            _a2b = load("a2b", a2tb, [128, E * 4 * 16])

            def _sl(tile_, idx, c):
                return tile_[:, idx * c:(idx + 1) * c]

            w_whh = lambda l, kt, ot: _sl(_whh, (l * KT + kt) * OT + ot, 128)
            w_wih = lambda l, kt, ot: _sl(_wih, (l * KT + kt) * OT + ot, 128)
            w_emb = lambda kt, mt: _sl(_emb, kt * 2 + mt, 128)
            w_za = lambda kt: _sl(_za, kt, B)
            w_g0 = lambda kt, mt: _sl(_g0, kt * 4 + mt, 128)
            w_g1 = lambda kt, mt: _sl(_g1, kt * 4 + mt, 128)
            w_g2r = lambda kt: _sl(_g2r, kt, 4)
            w_a0 = lambda e, kt, mt: _sl(_a0, (e * 2 + kt) * 4 + mt, 128)
            w_a1 = lambda e, kt, mt: _sl(_a1, (e * 4 + kt) * 4 + mt, 128)
            w_a2a = lambda e, kt: _sl(_a2a, e * 4 + kt, 128)
            w_a2b = lambda e, kt: _sl(_a2b, e * 4 + kt, 16)
            w_idm = load("idm", idm, [128, 128], f32)

            ones1 = sp.tile([1, 128], bf16, tag="ones1")
            nc.gpsimd.memset(ones1[:], 1.0)
            zero_h = sp.tile([128, 2 * B], bf16, tag="zero_h")
            nc.gpsimd.memset(zero_h[:], 0.0)

            # persistent activation state
            hist = [sp.tile([128, 32 * 2 * B], bf16, tag=f"hist{l}", name=f"hist{l}")
                    for l in range(3)]
            zfT = sp.tile([128, KT * NTOK], bf16, tag="zfT")
            # per-layer gi buffers, layout (parity, ot, s, b)
            gi_sb = [sp.tile([128, 2 * OT * CH * B], bf16, tag=f"gi{l}",
                             name=f"gi{l}") for l in range(L)]

            # ------------------------------------------------ embedding -> uT
            with tc.tile_pool(name="ps_pre", bufs=2, space="PSUM") as ppre:
                uT = sp.tile([128, KT * B], bf16, tag="uT")
                for mt in range(2):
                    pu = ppre.tile([128, B], f32, tag="pu")
                    for kt in range(3):
                        nc.tensor.matmul(pu[:], w_emb(kt, mt), w_za(kt),
                                         start=(kt == 0), stop=(kt == 2))
                    nc.scalar.copy(uT[:, mt * B:(mt + 1) * B], pu[:])

            # ------------------------------------------------ GRU waves
            with tc.tile_pool(name="ps_pg", bufs=1, space="PSUM") as ppg, \
                 tc.tile_pool(name="ps_gib", bufs=2, space="PSUM") as pgib:

                pg = [ppg.tile([128, OT * B], f32, tag=f"pg{l}", name=f"pg{l}")
                      for l in range(L)]

                def h_src(l, t, kt):
                    if t < 0:
                        return zero_h[:, kt * B:(kt + 1) * B]
                    if l < 3:
                        s = (t % 32) * 2 * B + kt * B
                        return hist[l][:, s:s + B]
                    return zfT[:, kt * NTOK + t * B: kt * NTOK + t * B + B]

                def h_prev_full(l, t):
                    # [128, 2, B] view of h_{t} (bf16)
                    if t < 0:
                        return zero_h[:].rearrange("p (k b) -> p k b", b=B)
                    if l < 3:
                        s = (t % 32) * 2 * B
                        return hist[l][:, s:s + 2 * B].rearrange(
                            "p (k b) -> p k b", b=B)
                    return zfT[:].rearrange("p (k n) -> p k n", k=KT)[
                        :, :, t * B:(t + 1) * B]

                def h_dst(l, t):
                    if l < 3:
                        s = (t % 32) * 2 * B
                        return hist[l][:, s:s + 2 * B].rearrange(
                            "p (k b) -> p k b", b=B)
                    return zfT[:].rearrange("p (k n) -> p k n", k=KT)[
                        :, :, t * B:(t + 1) * B]

                def emit_bulk_gi(l, c):
                    """Compute gi for (layer l, chunk c) into gi_sb[l] parity
                    half; src is layer l-1 history (or u_t for l=0)."""
                    par = c % 2
                    if l == 0:
                        # u_chunk[s, kt, b] = uT + t*utp  (2 DVE ops)
                        usc = wk.tile([128, CH * 2 * B], f32, tag="usc")
                        u3 = usc[:].rearrange("p (s z) -> p s z", z=2 * B)
                        tts = _tt[:, c * CH:(c + 1) * CH].unsqueeze(
                            2).broadcast_to((128, CH, 2 * B))
                        utv = _utp[:].unsqueeze(1).broadcast_to(
                            (128, CH, 2 * B))
                        nc.vector.tensor_tensor(u3, tts, utv, op=ALU.mult)
                        ub = wk.tile([128, CH * 2 * B], bf16, tag="ub")
                        ub3 = ub[:].rearrange("p (s z) -> p s z", z=2 * B)
                        ubv = uT[:].unsqueeze(1).broadcast_to((128, CH, 2 * B))
                        nc.vector.tensor_tensor(ub3, u3, ubv, op=ALU.add)
                        src = ub3
                    else:
                        src = hist[l - 1][:].rearrange(
                            "p (s z) -> p s z", z=2 * B)
                    s0 = 0 if l == 0 else (c * CH) % 32
                    for otp in range(3):          # ot pairs
                        pb = pgib.tile([128, 2 * CH * B], f32, tag="gib")
                        for oti in range(2):
                            ot = otp * 2 + oti
                            for kt in range(KT):
                                rhs = src[:, s0:s0 + CH, kt * B:(kt + 1) * B]
                                nc.tensor.matmul(
                                    pb[:, oti * CH * B:(oti + 1) * CH * B],
                                    w_wih(l, kt, ot), rhs, start=(kt == 0),
                                    stop=(kt == KT - 1))
                        dst = gi_sb[l][:, par * OT * CH * B + otp * 2 * CH * B:
                                       par * OT * CH * B + (otp + 1) * 2 * CH * B]
                        if otp == 0:
                            nc.scalar.copy(dst, pb[:])
                        else:
                            nc.vector.tensor_copy(dst, pb[:])

                def step_body(l, c, s):
                    t = c * CH + s
                    par = c % 2
                    g = gi_sb[l][:].rearrange(
                        "p (q o s b) -> p q o s b", q=2, o=OT, s=CH)
                    gi_rz = g[:, par, 0:4, s, :]
                    gi_n = g[:, par, 4:6, s, :]
                    pgl = pg[l]
                    # preload gi_rz into PSUM, accumulate gh on top
                    nc.tensor.matmul(pgl[:, 0:4 * B], _idmb[:], gi_rz,
                                     start=True, stop=False,
                                     skip_group_check=True)
                    for ot in range(4):
                        for kt in range(KT):
                            nc.tensor.matmul(
                                pgl[:, ot * B:(ot + 1) * B],
                                w_whh(l, kt, ot), h_src(l, t - 1, kt),
                                start=False, stop=(kt == KT - 1),
                                skip_group_check=True)
                    for ot in (4, 5):
                        for kt in range(KT):
                            nc.tensor.matmul(
                                pgl[:, ot * B:(ot + 1) * B],
                                w_whh(l, kt, ot), h_src(l, t - 1, kt),
                                start=(kt == 0), stop=(kt == KT - 1),
                                skip_group_check=True)
                    # r | v = sigmoid(rz)   (z rows negated => v = 1-u)
                    ru = wk.tile([128, 4 * B], f32, tag=f"ru{l}",
                                 name=f"ru{l}_{t}")
                    nc.scalar.activation(ru[:], pgl[:, 0:4 * B], AF.Sigmoid)
                    # hn = r * gh_n
                    hn = wk.tile([128, 2 * B], f32, tag=f"hn{l}",
                                 name=f"hn{l}_{t}")
                    nc.vector.tensor_tensor(hn[:], ru[:, 0:2 * B],
                                            pgl[:, 4 * B:6 * B], op=ALU.mult)
                    # n_in = hn + gi_n
                    n_in = wk.tile([128, 2 * B], f32, tag=f"ni{l}",
                                   name=f"ni{l}_{t}")
                    nc.gpsimd.tensor_tensor(
                        n_in[:].rearrange("p (o b) -> p o b", b=B),
                        hn[:].rearrange("p (o b) -> p o b", b=B),
                        gi_n, op=ALU.add)
                    nt = wk.tile([128, 2 * B], f32, tag=f"nt{l}",
                                 name=f"nt{l}_{t}")
                    nc.scalar.activation(nt[:], n_in[:], AF.Tanh)
                    # hmv = (v-1)*h_prev   (off the nt-chain)
                    hmv = wk.tile([128, 2 * B], f32, tag=f"hm{l}",
                                  name=f"hm{l}_{t}")
                    nc.vector.scalar_tensor_tensor(
                        hmv[:].rearrange("p (k b) -> p k b", b=B),
                        ru[:, 2 * B:4 * B].rearrange("p (k b) -> p k b", b=B),
                        1.0, h_prev_full(l, t - 1),
                        op0=ALU.subtract, op1=ALU.mult)
                    # w = v*nt ; h = w - hmv  -> bf16 state
                    w_ = wk.tile([128, 2 * B], f32, tag=f"w{l}",
                                 name=f"w{l}_{t}")
                    nc.vector.tensor_tensor(w_[:], ru[:, 2 * B:4 * B], nt[:],
                                            op=ALU.mult)
                    nc.gpsimd.tensor_tensor(
                        h_dst(l, t),
                        w_[:].rearrange("p (k b) -> p k b", b=B),
                        hmv[:].rearrange("p (k b) -> p k b", b=B),
                        op=ALU.subtract)

                for w in range(NCH + L - 1):
                    active = [(l, w - l) for l in range(L) if 0 <= w - l < NCH]
                    for (l, c) in active:
                        emit_bulk_gi(l, c)
                    for s in range(CH):
                        for (l, c) in active:
                            step_body(l, c, s)

            if debug:
                nc.sync.dma_start(dbg["dbg_uT"][:], uT[:])
                nc.sync.dma_start(dbg["dbg_zf"][:], zfT[:])
                nc.sync.dma_start(dbg["dbg_gi1"][:], gi_sb[1][:])

            # ------------------------------------------------ MoE chunks
            with tc.tile_pool(name="ps_gg", bufs=2, space="PSUM") as pgg, \
                 tc.tile_pool(name="ps_moe", bufs=4, space="PSUM") as pmo, \
                 tc.tile_pool(name="ps_sm", bufs=2, space="PSUM") as psm, \
                 tc.tile_pool(name="moeg", bufs=2) as mpg, \
                 tc.tile_pool(name="moe", bufs=1) as mp, \
                 tc.tile_pool(name="moesc", bufs=2) as msc:

                def elu1(dst, src_ps):
                    # elu(x) = relu(x) + (min(exp(x), 1) - 1); f32 until the
                    # final add.
                    tr = wk.tile([128, TOKC], f32, tag="elu_r")
                    nc.scalar.activation(tr[:], src_ps, AF.Relu)
                    te = wk.tile([128, TOKC], f32, tag="elu_e")
                    nc.scalar.activation(te[:], src_ps, AF.Exp)
                    nc.vector.tensor_scalar(te[:], te[:], 1.0, -1.0,
                                            op0=ALU.min, op1=ALU.add)
                    nc.vector.tensor_tensor(dst, tr[:], te[:], op=ALU.add)

                def zc(ctk, kt):
                    return zfT[:, kt * NTOK + ctk * TOKC:
                               kt * NTOK + (ctk + 1) * TOKC]

                def emit_gating(ctk):
                    """Gating tower for chunk ctk -> broadcast gate weights
                    gwb [128, E*TOKC].  Uses the 'gg'/'sm' PSUM tags only, so
                    it runs concurrently with the previous chunk's blends."""
                    g0o = mpg.tile([128, 4 * TOKC], bf16, tag="g0o")
                    for mt in range(4):
                        pb = pgg.tile([128, TOKC], f32, tag="gg")
                        for kt in range(KT):
                            nc.tensor.matmul(pb[:], w_g0(kt, mt), zc(ctk, kt),
                                             start=(kt == 0), stop=(kt == KT - 1))
                        elu1(g0o[:, mt * TOKC:(mt + 1) * TOKC], pb[:])
                    g1o = mpg.tile([128, 4 * TOKC], bf16, tag="g1o")
                    for mt in range(4):
                        pb = pgg.tile([128, TOKC], f32, tag="gg")
                        for kt in range(4):
                            nc.tensor.matmul(pb[:], w_g1(kt, mt),
                                             g0o[:, kt * TOKC:(kt + 1) * TOKC],
                                             start=(kt == 0), stop=(kt == 3))
                        elu1(g1o[:, mt * TOKC:(mt + 1) * TOKC], pb[:])
                    pg2 = psm.tile([128, 16], f32, tag="sm", name=f"pg2_{ctk}")
                    for tt in range(4):
                        for kt in range(4):
                            nc.tensor.matmul(
                                pg2[:, tt * 4:(tt + 1) * 4],
                                g1o[:, kt * TOKC + tt * 128: kt * TOKC + (tt + 1) * 128],
                                w_g2r(kt), start=(kt == 0), stop=(kt == 3),
                                skip_group_check=True)
                    g4 = lambda a: a.rearrange("p (g e) -> p g e", e=4)
                    mx = wk.tile([128, 4], f32, tag="sm_mx")
                    nc.vector.tensor_reduce(mx[:], g4(pg2[:]),
                                            axis=mybir.AxisListType.X,
                                            op=ALU.max)
                    sub = wk.tile([128, 16], f32, tag="sm_sub")
                    nc.vector.tensor_tensor(g4(sub[:]), g4(pg2[:]),
                                            mx[:].unsqueeze(2).broadcast_to(
                                                (128, 4, 4)), op=ALU.subtract)
                    ex = wk.tile([128, 16], f32, tag="sm_ex")
                    nc.scalar.activation(ex[:], sub[:], AF.Exp)
                    sm = wk.tile([128, 4], f32, tag="sm_sum")
                    nc.vector.tensor_reduce(sm[:], g4(ex[:]),
                                            axis=mybir.AxisListType.X, op=ALU.add)
                    rec = wk.tile([128, 4], f32, tag="sm_rec")
                    nc.vector.reciprocal(rec[:], sm[:])
                    gw = wk.tile([128, 16], f32, tag="sm_gw")
                    nc.vector.tensor_tensor(g4(gw[:]), g4(ex[:]),
                                            rec[:].unsqueeze(2).broadcast_to(
                                                (128, 4, 4)), op=ALU.mult)
                    # transpose gw -> [4, 512], then broadcast to [128, 512]
                    pgt = psm.tile([4, TOKC], f32, tag="sm", name=f"pgt_{ctk}")
                    for tt in range(4):
                        nc.tensor.transpose(pgt[:, tt * 128:(tt + 1) * 128],
                                            gw[:, tt * 4:(tt + 1) * 4], w_idm[:])
                    gwT = mpg.tile([4, TOKC], bf16, tag="gwT")
                    nc.scalar.copy(gwT[:], pgt[:])
                    gwf = mpg.tile([1, E * TOKC], bf16, tag="gwf")
                    nc.sync.dma_start(
                        gwf[:].rearrange("p (e n) -> p e n", e=E), gwT[:])
                    gwb = mpg.tile([128, E * TOKC], bf16, tag="gwb")
                    for e in range(E):
                        pbc = pgg.tile([128, TOKC], f32, tag="gg",
                                       name=f"pbc{ctk}_{e}")
                        nc.tensor.matmul(pbc[:], ones1[:],
                                         gwf[0:1, e * TOKC:(e + 1) * TOKC],
                                         start=True, stop=True)
                        nc.scalar.copy(gwb[:, e * TOKC:(e + 1) * TOKC], pbc[:])
                    if debug and ctk == 0:
                        nc.sync.dma_start(dbg["dbg_gw"][:], gw[:])
                        nc.sync.dma_start(dbg["dbg_g0o"][:], g0o[:])
                    return gwb

                def emit_blends(ctk, gwb):
                    # blend 0: inputs zc (2 k-tiles), out 512
                    pbs = [pmo.tile([128, TOKC], f32, tag="big", name=f"pbs{_i}")
                           for _i in range(4)]
                    xsc = msc.tile([128, KT * TOKC], bf16, tag="hsc")
                    for e in range(E):
                        for kt in range(KT):
                            nc.vector.tensor_tensor(
                                xsc[:, kt * TOKC:(kt + 1) * TOKC], zc(ctk, kt),
                                gwb[:, e * TOKC:(e + 1) * TOKC], op=ALU.mult)
                        for mt in range(4):
                            for kt in range(KT):
                                nc.tensor.matmul(
                                    pbs[mt][:], w_a0(e, kt, mt),
                                    xsc[:, kt * TOKC:(kt + 1) * TOKC],
                                    start=(e == 0 and kt == 0),
                                    stop=(e == 3 and kt == KT - 1),
                                    skip_group_check=True)
                    h1o = mp.tile([128, 4 * TOKC], bf16, tag="h1o")
                    for mt in range(4):
                        elu1(h1o[:, mt * TOKC:(mt + 1) * TOKC], pbs[mt][:])
                    if debug and ctk == 0:
                        nc.sync.dma_start(dbg["dbg_h1o"][:], h1o[:])

                    # blend 1: inputs h1o (4 k-tiles)
                    pbs = [pmo.tile([128, TOKC], f32, tag="big", name=f"pbs{_i}")
                           for _i in range(4)]
                    h1sc = msc.tile([128, 4 * TOKC], bf16, tag="hsc")
                    for e in range(E):
                        for kt in range(4):
                            nc.vector.tensor_tensor(
                                h1sc[:, kt * TOKC:(kt + 1) * TOKC],
                                h1o[:, kt * TOKC:(kt + 1) * TOKC],
                                gwb[:, e * TOKC:(e + 1) * TOKC], op=ALU.mult)
                        for mt in range(4):
                            for kt in range(4):
                                nc.tensor.matmul(
                                    pbs[mt][:], w_a1(e, kt, mt),
                                    h1sc[:, kt * TOKC:(kt + 1) * TOKC],
                                    start=(e == 0 and kt == 0),
                                    stop=(e == 3 and kt == 3),
                                    skip_group_check=True)
                    h2o = mp.tile([128, 4 * TOKC], bf16, tag="h2o")
                    for mt in range(4):
                        elu1(h2o[:, mt * TOKC:(mt + 1) * TOKC], pbs[mt][:])
                    # blend 2: out 144 = 128 + 16
                    poa = pmo.tile([128, TOKC], f32, tag="big")
                    pob = psm.tile([16, TOKC], f32, tag="sm", name=f"pob_{ctk}")
                    h2sc = msc.tile([128, 4 * TOKC], bf16, tag="hsc")
                    for e in range(E):
                        for kt in range(4):
                            nc.vector.tensor_tensor(
                                h2sc[:, kt * TOKC:(kt + 1) * TOKC],
                                h2o[:, kt * TOKC:(kt + 1) * TOKC],
                                gwb[:, e * TOKC:(e + 1) * TOKC], op=ALU.mult)
                        for kt in range(4):
                            last = (e == 3 and kt == 3)
                            nc.tensor.matmul(poa[:], w_a2a(e, kt),
                                             h2sc[:, kt * TOKC:(kt + 1) * TOKC],
                                             start=(e == 0 and kt == 0), stop=last,
                                             skip_group_check=True)
                            nc.tensor.matmul(pob[:], w_a2b(e, kt),
                                             h2sc[:, kt * TOKC:(kt + 1) * TOKC],
                                             start=(e == 0 and kt == 0), stop=last,
                                             skip_group_check=True)
                    oa = mp.tile([128, TOKC], f32, tag="oa")
                    nc.scalar.copy(oa[:], poa[:])
                    ob = mp.tile([16, TOKC], f32, tag="ob")
                    nc.scalar.copy(ob[:], pob[:])
                    # out[o, t, b]: src [o_part, (t 32, b 16)] - both contiguous
                    t0 = ctk * (TOKC // B)
                    nc.sync.dma_start(out_d[0:128, t0:t0 + 32, :], oa[:].rearrange(
                        "p (t b) -> p t b", b=B))
                    nc.sync.dma_start(out_d[128:144, t0:t0 + 32, :], ob[:].rearrange(
                        "p (t b) -> p t b", b=B))

                gwb_cur = emit_gating(0)
                for ctk in range(NMC):
                    gwb_next = emit_gating(ctk + 1) if ctk + 1 < NMC else None
                    emit_blends(ctk, gwb_cur)
                    gwb_cur = gwb_next
    return nc


# ------------------------------------------------------------- walrus fixup
def _fix_sync_waits(nc, max_waits=1):
    """This walrus build allows only one sync wait per instruction; move
    excess waits onto NOPs inserted ahead of the instruction."""
    import concourse.mybir as mybir
    import bass_rust
    ctr = 0
    for f in nc.m.functions:
        for blk in f.blocks:
            out = []
            changed = False
            for inst in blk.instructions:
                si = inst.sync_info
                if si is not None and si.on_wait and len(si.on_wait) > max_waits:
                    waits = list(si.on_wait)
                    extra, keep = waits[:-max_waits], waits[-max_waits:]
                    for w_ in extra:
                        ctr += 1
                        nop = mybir.InstNoOp(name=f"WSPLIT-{ctr}", ins=[], outs=[])
                        nop.engine = inst.engine
                        nop.sync_info = bass_rust.SyncInfo(on_wait=[w_], on_update=[])
                        out.append(nop)
                    inst.sync_info = bass_rust.SyncInfo(
                        on_wait=keep, on_update=list(si.on_update))
                    changed = True
                out.append(inst)
            if changed:
                blk.instructions = out
    return ctr


# ------------------------------------------------------------- preprocessing
def _prep_core_inputs(inputs, T_=T):
    z = _f32(inputs["z"])
    y = np.asarray(inputs["y"]).astype(np.int64)
    lengths = np.asarray(inputs["lengths"]).astype(np.float64)
    emb_w = _f32(inputs["emb_w"])      # [H, D+NC+1]
    gru_wih = _f32(inputs["gru_wih"]).copy()  # [4, 3H, H]
    gru_whh = _f32(inputs["gru_whh"]).copy()
    g0_w = _f32(inputs["g0_w"]); g1_w = _f32(inputs["g1_w"]); g2_w = _f32(inputs["g2_w"])
    g0_b = _f32(inputs["g0_b"]); g1_b = _f32(inputs["g1_b"]); g2_b = _f32(inputs["g2_b"])
    a0 = _f32(inputs["alpha0"]); a1 = _f32(inputs["alpha1"]); a2 = _f32(inputs["alpha2"])
    b0 = _f32(inputs["beta0"]); b1 = _f32(inputs["beta1"]); b2 = _f32(inputs["beta2"])
    emb_b = _f32(inputs["emb_b"])
    bsum = _f32(inputs["gru_bih"]) + _f32(inputs["gru_bhh"])  # [4, 3H] assumed zero

    # negate z-gate rows so sigmoid yields v = 1-u directly
    gru_wih[:, H:2 * H, :] *= -1.0
    gru_whh[:, H:2 * H, :] *= -1.0

    # ---- shared (replicated) tensors
    def pack_lhsT(w, cols=128):
        # w: [O, K]; lhsT = w.T tiled [K//128, O//cols, 128, cols]
        # -> flat [128, ntiles*cols], tile index = kt*OT_ + ot (kt-major)
        O, K = w.shape
        ktn, otn = K // 128, O // cols
        wt = np.ascontiguousarray(w.T).reshape(ktn, 128, otn, cols)
        return wt.transpose(1, 0, 2, 3).reshape(128, ktn * otn * cols)

    whh_t = _bf16(np.concatenate([pack_lhsT(gru_whh[l]) for l in range(4)], axis=1))
    wih_t = _bf16(np.concatenate([pack_lhsT(gru_wih[l]) for l in range(4)], axis=1))
    embT = np.zeros((256, 384), np.float32)
    embT[:, :269] = emb_w
    emb_t = _f32(pack_lhsT(embT))                 # [128, 3kt*2mt*128]
    g0t = _bf16(pack_lhsT(g0_w))
    g1t = _bf16(pack_lhsT(g1_w))
    # g2 rhs tiles: g2_w.T [512, 4] -> [4kt][128, 4] -> [128, 16]
    g2r = _bf16(np.ascontiguousarray(g2_w.T).reshape(4, 128, 4)
                .transpose(1, 0, 2).reshape(128, 16))
    a0t = _bf16(np.concatenate([pack_lhsT(a0[e]) for e in range(E)], axis=1))
    a1t = _bf16(np.concatenate([pack_lhsT(a1[e]) for e in range(E)], axis=1))
    # alpha2: [E, 144, 512]: lhsT [512, 144] -> cols 0:128 and 128:144
    a2T = np.stack([np.ascontiguousarray(a2[e].T) for e in range(E)])  # [E,512,144]
    a2r = a2T.reshape(E, 4, 128, 144)
    a2ta = _bf16(a2r[:, :, :, 0:128].transpose(2, 0, 1, 3).reshape(128, E * 4 * 128))
    a2tb = _bf16(a2r[:, :, :, 128:144].transpose(2, 0, 1, 3).reshape(128, E * 4 * 16))
    idm = _f32(np.eye(128))
    idmb = _bf16(np.eye(128))
    tt = _f32(np.broadcast_to(np.arange(T_, dtype=np.float32), (128, T_)))
    # NOTE: biases (g0_b/g1_b/g2_b, emb_b, gru biases) and betas are zero by
    # the input spec; they are asserted here so a nonzero case fails loudly.
    for _nm, _v in [("g0_b", g0_b), ("g1_b", g1_b), ("g2_b", g2_b),
                    ("emb_b", emb_b), ("beta0", b0), ("beta1", b1),
                    ("beta2", b2), ("gru_b", bsum)]:
        assert np.abs(_v).max() == 0.0, f"{_nm} nonzero; kernel assumes zero"

    # ---- per-core tensors
    yoh = np.zeros((BS, NCLS), np.float32)
    yoh[np.arange(BS), y] = 1.0
    za = np.concatenate([z, yoh], axis=1)              # [BS, 268]
    za_pad = np.zeros((BS, 384), np.float32)
    za_pad[:, :268] = za
    # feature 268 (time) enters via utp, not za.
    cb = 1.0 / (lengths - 1.0)
    u_time = emb_w[:, 268]                             # [256]

    maps = []
    for cidx in range(NCORES):
        sl = slice(cidx * B, (cidx + 1) * B)
        za_c = np.ascontiguousarray(za_pad[sl].T)      # [384, B]
        za_t = _f32(za_c.reshape(3, 128, B).transpose(1, 0, 2).reshape(128, 3 * B))
        # utp[p, kt, b] = u_time[kt*128+p] * cb[b]
        utp_v = np.einsum("f,b->fb", u_time, cb[sl])   # [256, B]
        utp = _f32(utp_v.reshape(2, 128, B).transpose(1, 0, 2).reshape(128, 2 * B))
        maps.append(dict(
            whh_t=whh_t, wih_t=wih_t, emb_t=emb_t, za_t=za_t,
            utp=utp, tt=tt, idmb=idmb, g0t=g0t, g1t=g1t, g2r=g2r,
            a0t=a0t, a1t=a1t, a2ta=a2ta, a2tb=a2tb, idm=idm,
        ))
    return maps


# ------------------------------------------------------------------ runner
def _get_runner(T_=T):
    key = ("runner", T_)
    if key in _STATE:
        return _STATE[key]
    import jax
    from jax.sharding import Mesh, PartitionSpec
    try:
        from jax.experimental.shard_map import shard_map
    except ImportError:
        from jax.shard_map import shard_map
    import concourse.mybir as mybir
    from concourse import bass2jax

    nc = _build_nc(T_)
    _fix_sync_waits(nc)
    bass2jax.install_neuronx_cc_hook()
    partition_name = nc.partition_id_tensor.name if nc.partition_id_tensor else None
    in_names, out_names, out_avals = [], [], []
    for alloc in nc.m.functions[0].allocations:
        if not isinstance(alloc, mybir.MemoryLocationSet):
            continue
        name = alloc.memorylocations[0].name
        if alloc.kind == "ExternalInput":
            if name != partition_name:
                in_names.append(name)
        elif alloc.kind == "ExternalOutput":
            out_names.append(name)
            out_avals.append(jax.core.ShapedArray(
                tuple(alloc.tensor_shape), mybir.dt.np(alloc.dtype)))
    n_params = len(in_names)
    all_names = in_names + out_names + ([partition_name] if partition_name else [])

    def _body(*args):
        operands = list(args)
        if partition_name is not None:
            operands.append(bass2jax.partition_id_tensor())
        return tuple(bass2jax._bass_exec_p.bind(
            *operands, out_avals=tuple(out_avals), in_names=tuple(all_names),
            out_names=tuple(out_names), lowering_input_output_aliases=(),
            sim_require_finite=True, sim_require_nnan=True, nc=nc))

    devices = jax.devices()[:NCORES]
    mesh = Mesh(np.asarray(devices), ("core",))
    n_outs = len(out_names)
    sharded = jax.jit(
        shard_map(_body, mesh=mesh,
                  in_specs=(PartitionSpec("core"),) * (n_params + n_outs),
                  out_specs=(PartitionSpec("core"),) * n_outs),
        keep_unused=True)
    runner = dict(sharded=sharded, in_names=in_names, out_names=out_names,
                  out_avals=out_avals)
    _STATE[key] = runner
    return runner


def _run_device(maps, T_=T, timing=None):
    import jax
    r = _get_runner(T_)
    concat_in = [np.concatenate([np.asarray(maps[c][n]) for c in range(NCORES)],
                                axis=0) for n in r["in_names"]]
    zeros = [np.zeros((NCORES * a.shape[0], *a.shape[1:]), a.dtype)
             for a in r["out_avals"]]
    din = [jax.device_put(x) for x in concat_in]
    dz = [jax.device_put(z) for z in zeros]
    out = r["sharded"](*din, *dz)
    jax.block_until_ready(out)
    if timing is not None:
        import time
        for _ in range(timing.get("iters", 10)):
            t0 = time.perf_counter()
            out = r["sharded"](*din, *dz)
            jax.block_until_ready(out)
            timing.setdefault("times", []).append(time.perf_counter() - t0)
    o = np.asarray(out[0])
    per_core = o.reshape(NCORES, OUTD, T_, B)
    return per_core


def kernel(**inputs):
    maps = _prep_core_inputs(inputs, T)
    per_core = _run_device(maps, T)          # [NCORES, OUTD, T, B]
    full = per_core.transpose(0, 3, 1, 2)    # [NCORES, B, OUTD, T]
    full = full.reshape(BS, NJ, NF, T)
    return np.ascontiguousarray(full.astype(np.float32))
